# revision 8
# baseline (speedup 1.0000x reference)
"""Trainium2 Bass kernel for nn_BGraphConvolution (BGCN message passing).

Sharding: nodes (rows of x / output) split across 8 NeuronCores (12500 each).
Each adjacency's edges are partitioned by destination row; per 128-dest tile
the edges are col-sorted, chunked into 128-edge chunks, and chunks grouped by
GCH so each group's source span stays < 32768 (int16 dma_gather indices).
The [N,128] pre_sup / bilinear difference matrices are AllGathered in bf16
(in two row-chunks, overlapped with compute) before each SpMM stage.
SpMM = dma_gather of source rows (4 SWDGE queues) + iota-compare one-hot
scatter matrix + bf16 TensorE matmuls accumulating in PSUM.  For supports
1-4 the spmm(X) and spmm(X^2) share one N=256 matmul stream per chunk
(gathered rows and their squares interleaved in one SBUF tile).
"""
import numpy as np
import ml_dtypes

N = 100000
D_IN, D_OUT = 256, 128
NCORE = 8
NSH = N // NCORE          # 12500 rows per core
P = 128
NT = (NSH + P - 1) // P   # 98 dest tiles per core (last has 84 rows)
LAST_ROWS = NSH - (NT - 1) * P
GCH = 8                   # chunks (of 128 edges) per gather group
MAX_SPAN = 32768
NQ = 4                    # SWDGE queues
HALF_T = 49               # AG chunk boundary (tiles 0..48 | 49..97)

bf16 = ml_dtypes.bfloat16


def _sort_tiles(rows_l, cols_l, vals_l):
    """Per dest tile: edges sorted by col. Returns [(tc, tr, tv)] * NT."""
    order = np.argsort(rows_l, kind="stable")
    r, c, v = rows_l[order], cols_l[order], vals_l[order]
    offs = np.concatenate([[0], np.cumsum(np.bincount(r // P, minlength=NT))])
    out = []
    for t in range(NT):
        s_, e_ = offs[t], offs[t + 1]
        tc, tr, tv = c[s_:e_], r[s_:e_] - t * P, v[s_:e_]
        o2 = np.argsort(tc, kind="stable")
        out.append((tc[o2], tr[o2], tv[o2]))
    return out


def _build_program(NCH, GR, idx_w, ch_w, max_nch, fused, chunked_ag):
    import concourse.bass as bass
    import concourse.tile as tile
    from concourse import bacc, mybir, library_config
    from concourse.masks import make_identity
    from contextlib import ExitStack

    fp32 = mybir.dt.float32
    bft = mybir.dt.bfloat16
    KCH = D_IN // P  # 2

    nc = bacc.Bacc("TRN2", target_bir_lowering=False, debug=False,
                   num_devices=NCORE, num_swdge_queues=NQ)
    xt_d = nc.dram_tensor("xt", [D_IN, NSH], fp32, kind="ExternalInput").ap()
    wa_d = nc.dram_tensor("wa", [D_IN, D_OUT], fp32, kind="ExternalInput").ap()
    wb_d = nc.dram_tensor("wb", [D_IN, D_OUT], fp32, kind="ExternalInput").ap()
    w1_d = nc.dram_tensor("w1", [D_OUT, 32], fp32, kind="ExternalInput").ap()
    b1_d = nc.dram_tensor("b1", [1, 32], fp32, kind="ExternalInput").ap()
    w2_d = nc.dram_tensor("w2", [32, 1], fp32, kind="ExternalInput").ap()
    iota_d = nc.dram_tensor("iota", [P, P], bft, kind="ExternalInput").ap()
    idx_d = nc.dram_tensor("idxm", [P, idx_w], mybir.dt.int16, kind="ExternalInput").ap()
    rlv_d = nc.dram_tensor("rlvm", [P, ch_w], fp32, kind="ExternalInput").ap()
    out_d = nc.dram_tensor("out", [NSH, D_OUT], fp32, kind="ExternalOutput").ap()

    qctr = [0]

    def next_q():
        q = qctr[0] % NQ
        qctr[0] += 1
        return q

    # meta offsets per (support, tile)
    off_idx, off_ch = [], []
    io, co = 0, 0
    for s in range(7):
        t_idx, t_ch = [], []
        for t in range(NT):
            t_idx.append(io)
            t_ch.append(co)
            io += NCH[s][t] * 8
            co += 2 * NCH[s][t]
        off_idx.append(t_idx)
        off_ch.append(t_ch)
    assert io == idx_w and co == ch_w, (io, idx_w, co, ch_w)

    rg = [list(range(NCORE))]

    with tile.TileContext(nc) as tc, ExitStack() as ctx:
        const_pool = ctx.enter_context(tc.tile_pool(name="const", bufs=1))
        meta_pool = ctx.enter_context(tc.tile_pool(name="meta", bufs=4))
        g_pool = ctx.enter_context(tc.tile_pool(name="g", bufs=4))
        s_pool = ctx.enter_context(tc.tile_pool(name="s", bufs=8))
        o_pool = ctx.enter_context(tc.tile_pool(name="o", bufs=3))
        dram = ctx.enter_context(tc.tile_pool(name="dram", bufs=1, space="DRAM"))

        nc.gpsimd.load_library(library_config.mlp)

        iota_t = const_pool.tile([P, P], bft)
        nc.sync.dma_start(iota_t[:], iota_d[:])
        ident = const_pool.tile([P, P], fp32)
        make_identity(nc, ident[:])
        wa_t = const_pool.tile([P, KCH * D_OUT], fp32, tag="wa")
        wb_t = const_pool.tile([P, KCH * D_OUT], fp32, tag="wb")
        for k in range(KCH):
            nc.sync.dma_start(wa_t[:, k * D_OUT:(k + 1) * D_OUT],
                              wa_d[k * P:(k + 1) * P, :])
            nc.sync.dma_start(wb_t[:, k * D_OUT:(k + 1) * D_OUT],
                              wb_d[k * P:(k + 1) * P, :])
        w1_t = const_pool.tile([P, 32], fp32)
        nc.sync.dma_start(w1_t[:], w1_d[:])
        b1_t = const_pool.tile([1, 32], fp32)
        nc.sync.dma_start(b1_t[:], b1_d[:])
        w2_t = const_pool.tile([32, 1], fp32)
        nc.sync.dma_start(w2_t[:], w2_d[:])
        ones_t = const_pool.tile([1, P], fp32)
        nc.vector.memset(ones_t[:], 1.0)
        s0_sb = const_pool.tile([P, NT * P], bft, tag="s0")

        p_local = dram.tile([NSH, D_OUT], bft, tag="p_local")
        d1_local = dram.tile([NSH, D_OUT], bft, tag="d1_local")
        d2_local = dram.tile([NSH, D_OUT], bft, tag="d2_local")
        p_full = dram.tile([N, D_OUT], bft, tag="p_full", addr_space="Shared")
        d1_full = dram.tile([N, D_OUT], bft, tag="d1_full", addr_space="Shared")
        d2_full = dram.tile([N, D_OUT], bft, tag="d2_full", addr_space="Shared")

        def ag(loc, full, lo, hi):
            if lo == 0 and hi == NSH:
                nc.gpsimd.collective_compute(
                    "AllGather", mybir.AluOpType.bypass, replica_groups=rg,
                    ins=[loc[:]], outs=[full[:]])
            else:
                view = full[:].rearrange("(c r) d -> c r d", c=NCORE)[:, lo:hi, :]
                nc.gpsimd.collective_compute(
                    "AllGather", mybir.AluOpType.bypass, replica_groups=rg,
                    ins=[loc[lo:hi, :]], outs=[view])

        # ---------- dense phase: pre_sup (transposed orientation) ----------
        with tc.tile_pool(name="dense", bufs=2) as dense_pool, \
             tc.tile_pool(name="dpsum", bufs=1, space="PSUM") as dpsum_pool:
            for t in range(NT):
                rows = P if t < NT - 1 else LAST_ROWS
                sl = slice(t * P, t * P + rows)
                xt_t = dense_pool.tile([P, KCH * P], fp32, tag="xt")
                for k in range(KCH):
                    nc.sync.dma_start(xt_t[:, k * P:k * P + rows],
                                      xt_d[k * P:(k + 1) * P, sl])
                psa = dpsum_pool.tile([P, P], fp32, tag="pa")
                psb = dpsum_pool.tile([P, P], fp32, tag="pb")
                for k in range(KCH):
                    nc.tensor.matmul(psa[:, :rows],
                                     lhsT=wa_t[:, k * D_OUT:(k + 1) * D_OUT],
                                     rhs=xt_t[:, k * P:k * P + rows],
                                     start=(k == 0), stop=(k == KCH - 1))
                    nc.tensor.matmul(psb[:, :rows],
                                     lhsT=wb_t[:, k * D_OUT:(k + 1) * D_OUT],
                                     rhs=xt_t[:, k * P:k * P + rows],
                                     start=(k == 0), stop=(k == KCH - 1))
                a_sb = dense_pool.tile([P, P], fp32, tag="a_sb")
                nc.vector.tensor_copy(a_sb[:, :rows], psa[:, :rows])
                tmp = dense_pool.tile([P, P], fp32, tag="tmp")
                nc.vector.tensor_tensor(out=tmp[:, :rows], in0=a_sb[:, :rows],
                                        in1=psb[:, :rows],
                                        op=mybir.AluOpType.subtract)
                nc.vector.tensor_tensor(out=tmp[:, :rows], in0=tmp[:, :rows],
                                        in1=a_sb[:, :rows],
                                        op=mybir.AluOpType.mult)
                al_sb = dense_pool.tile([P, P], fp32, tag="al_sb")
                nc.vector.tensor_scalar(out=al_sb[:, :rows], in0=tmp[:, :rows],
                                        scalar1=0.5, scalar2=None,
                                        op0=mybir.AluOpType.mult)
                nc.vector.tensor_tensor(out=al_sb[:, :rows],
                                        in0=al_sb[:, :rows],
                                        in1=a_sb[:, :rows],
                                        op=mybir.AluOpType.add)
                z = []
                for zi, comp_sb in enumerate((a_sb, al_sb)):
                    psh = dpsum_pool.tile([32, P], fp32, tag="ph")
                    nc.tensor.matmul(psh[:, :rows], lhsT=w1_t[:],
                                     rhs=comp_sb[:, :rows],
                                     start=True, stop=False)
                    nc.tensor.matmul(psh[:, :rows], lhsT=b1_t[:],
                                     rhs=ones_t[:, :rows],
                                     start=False, stop=True)
                    h_sb = dense_pool.tile([32, P], fp32, tag="h_sb")
                    nc.scalar.activation(h_sb[:, :rows], psh[:, :rows],
                                         mybir.ActivationFunctionType.Tanh)
                    psz = dpsum_pool.tile([1, P], fp32, tag="pz")
                    nc.tensor.matmul(psz[:, :rows], lhsT=w2_t[:],
                                     rhs=h_sb[:, :rows], start=True, stop=True)
                    z_sb = dense_pool.tile([1, P], fp32, tag=f"z{zi}")
                    nc.vector.tensor_copy(z_sb[:, :rows], psz[:, :rows])
                    z.append(z_sb)
                dz = dense_pool.tile([1, P], fp32, tag="dz")
                nc.vector.tensor_tensor(out=dz[:, :rows], in0=z[1][:, :rows],
                                        in1=z[0][:, :rows],
                                        op=mybir.AluOpType.subtract)
                ez = dense_pool.tile([1, P], fp32, tag="ez")
                nc.scalar.activation(ez[:, :rows], dz[:, :rows],
                                     mybir.ActivationFunctionType.Exp)
                nc.vector.tensor_scalar(out=ez[:, :rows], in0=ez[:, :rows],
                                        scalar1=1.0, scalar2=None,
                                        op0=mybir.AluOpType.add)
                atta = dense_pool.tile([1, P], fp32, tag="atta")
                nc.vector.reciprocal(atta[:, :rows], ez[:, :rows])
                attb = dense_pool.tile([P, P], fp32, tag="attb")
                nc.gpsimd.partition_broadcast(attb[:, :rows], atta[:, :rows])
                t1 = dense_pool.tile([P, P], fp32, tag="t1")
                nc.vector.tensor_tensor(out=t1[:, :rows], in0=a_sb[:, :rows],
                                        in1=attb[:, :rows],
                                        op=mybir.AluOpType.mult)
                attb2 = dense_pool.tile([P, P], fp32, tag="attb2")
                nc.vector.tensor_scalar(out=attb2[:, :rows],
                                        in0=attb[:, :rows],
                                        scalar1=-1.0, scalar2=1.0,
                                        op0=mybir.AluOpType.mult,
                                        op1=mybir.AluOpType.add)
                t2 = dense_pool.tile([P, P], fp32, tag="t2")
                nc.vector.tensor_tensor(out=t2[:, :rows], in0=al_sb[:, :rows],
                                        in1=attb2[:, :rows],
                                        op=mybir.AluOpType.mult)
                pst = dense_pool.tile([P, P], fp32, tag="pst")
                nc.vector.tensor_tensor(out=pst[:, :rows], in0=t1[:, :rows],
                                        in1=t2[:, :rows],
                                        op=mybir.AluOpType.add)
                ptp = dpsum_pool.tile([P, P], fp32, tag="ptp")
                nc.tensor.transpose(out=ptp[:rows, :], in_=pst[:, :rows],
                                    identity=ident[:])
                prow = dense_pool.tile([P, P], bft, tag="prow")
                nc.vector.tensor_copy(prow[:rows, :], ptp[:rows, :])
                nc.sync.dma_start(p_local[sl, :], prow[:rows, :])
                if chunked_ag and t == HALF_T - 1:
                    ag(p_local, p_full, 0, HALF_T * P)
            if chunked_ag:
                ag(p_local, p_full, HALF_T * P, NSH)
            else:
                ag(p_local, p_full, 0, NSH)

        def spmm_tile(s, t, src_full, psum, want_q, first, last, sq_eng):
            nch_t = NCH[s][t]
            ioff = off_idx[s][t]
            coff = off_ch[s][t]
            idxt = meta_pool.tile([P, max_nch * 8], mybir.dt.int16, tag="idxt")
            rlvt = meta_pool.tile([P, 2 * max_nch], fp32, tag="rlvt")
            nc.sync.dma_start(idxt[:, :nch_t * 8], idx_d[:, ioff:ioff + nch_t * 8])
            nc.sync.dma_start(rlvt[:, :2 * nch_t], rlv_d[:, coff:coff + 2 * nch_t])
            gt = g_pool.tile([P, 2 * max_nch * P], bft, tag="G")
            g4 = gt[:].rearrange("p (h c d) -> p h c d", h=2, d=P)
            off = 0
            for (b0, span, cnt) in GR[s][t]:
                nc.gpsimd.dma_gather(
                    out_ap=g4[:, 0, off:off + cnt, :],
                    in_ap=src_full[b0:b0 + span, :],
                    idxs_ap=idxt[:, off * 8:(off + cnt) * 8],
                    num_idxs=cnt * P, num_idxs_reg=cnt * P, elem_size=D_OUT,
                    single_packet=False, queue_num=next_q(),
                )
                off += cnt
            if want_q:
                if sq_eng == "act":
                    nc.scalar.square(g4[:, 1, :nch_t, :], g4[:, 0, :nch_t, :])
                else:
                    nc.vector.tensor_tensor(out=g4[:, 1, :nch_t, :],
                                            in0=g4[:, 0, :nch_t, :],
                                            in1=g4[:, 0, :nch_t, :],
                                            op=mybir.AluOpType.mult)
            for c in range(nch_t):
                s_t = s_pool.tile([P, P], bft)
                nc.vector.tensor_scalar(
                    out=s_t[:], in0=iota_t[:],
                    scalar1=rlvt[:, c:c + 1],
                    scalar2=rlvt[:, nch_t + c:nch_t + c + 1],
                    op0=mybir.AluOpType.is_equal, op1=mybir.AluOpType.mult)
                first_c = first and c == 0
                last_c = last and c == nch_t - 1
                if want_q and fused:
                    nc.tensor.matmul(psum[:, :2 * P], lhsT=s_t[:],
                                     rhs=g4[:, :, c, :], start=first_c,
                                     stop=last_c, skip_group_check=True)
                else:
                    nc.tensor.matmul(psum[:, :P], lhsT=s_t[:],
                                     rhs=g4[:, 0, c, :], start=first_c,
                                     stop=last_c, skip_group_check=True)
                    if want_q:
                        nc.tensor.matmul(psum[:, P:2 * P], lhsT=s_t[:],
                                         rhs=g4[:, 1, c, :], start=first_c,
                                         stop=last_c, skip_group_check=True)

        # ---------- supports 1-4 -> d1, d2 (fused s|q matmul streams) ------
        with tc.tile_pool(name="psB", bufs=2, space="PSUM") as psB:
            for t in range(NT):
                rows = P if t < NT - 1 else LAST_ROWS
                sl = slice(t * P, t * P + rows)
                ps = {}
                for s in (1, 2, 3, 4):
                    p_sq = psB.tile([P, 2 * P], fp32, tag=f"ps{s}")
                    spmm_tile(s, t, p_full, p_sq, True, True, True,
                              "act" if s == 2 else "dve")
                    ps[s] = p_sq
                for dloc, (sa, sb_) in ((d1_local, (1, 3)), (d2_local, (2, 4))):
                    pA, pB = ps[sa], ps[sb_]
                    tA = o_pool.tile([P, P], fp32, tag="tA")
                    nc.vector.tensor_tensor(out=tA[:], in0=pA[:, :P],
                                            in1=pA[:, :P],
                                            op=mybir.AluOpType.mult)
                    nc.vector.tensor_tensor(out=tA[:], in0=tA[:],
                                            in1=pA[:, P:],
                                            op=mybir.AluOpType.subtract)
                    tB = o_pool.tile([P, P], fp32, tag="tB")
                    nc.vector.tensor_tensor(out=tB[:], in0=pB[:, :P],
                                            in1=pB[:, :P],
                                            op=mybir.AluOpType.mult)
                    nc.vector.tensor_tensor(out=tB[:], in0=tB[:],
                                            in1=pB[:, P:],
                                            op=mybir.AluOpType.subtract)
                    dd = o_pool.tile([P, P], bft, tag="dd")
                    nc.vector.tensor_tensor(out=dd[:], in0=tA[:], in1=tB[:],
                                            op=mybir.AluOpType.subtract)
                    nc.sync.dma_start(dloc[sl, :], dd[:rows, :])
                if chunked_ag and t == HALF_T - 1:
                    ag(d1_local, d1_full, 0, HALF_T * P)
                    ag(d2_local, d2_full, 0, HALF_T * P)
            if chunked_ag:
                ag(d1_local, d1_full, HALF_T * P, NSH)
                ag(d2_local, d2_full, HALF_T * P, NSH)
            else:
                ag(d1_local, d1_full, 0, NSH)
                ag(d2_local, d2_full, 0, NSH)

        # ---------- support 0 -> s0 (SBUF), overlaps the d1/d2 AllGather ---
        with tc.tile_pool(name="ps0", bufs=4, space="PSUM") as ps0_pool:
            for t in range(NT):
                p0 = ps0_pool.tile([P, P], fp32, tag="p0")
                spmm_tile(0, t, p_full, p0, False, True, True, None)
                nc.vector.tensor_copy(s0_sb[:, t * P:(t + 1) * P], p0[:])

        # ---------- final: out = relu(s0 + spmm5'(D1) + spmm6'(D2)) --------
        with tc.tile_pool(name="psC", bufs=4, space="PSUM") as psC:
            for t in range(NT):
                rows = P if t < NT - 1 else LAST_ROWS
                sl = slice(t * P, t * P + rows)
                psum_f = psC.tile([P, P], fp32, tag="pf")
                spmm_tile(5, t, d1_full, psum_f, False, True, False, None)
                spmm_tile(6, t, d2_full, psum_f, False, False, True, None)
                o1 = o_pool.tile([P, P], fp32, tag="o1")
                nc.vector.tensor_tensor(out=o1[:], in0=psum_f[:],
                                        in1=s0_sb[:, t * P:(t + 1) * P],
                                        op=mybir.AluOpType.add)
                o2 = o_pool.tile([P, P], fp32, tag="o2")
                nc.scalar.activation(o2[:], o1[:],
                                     mybir.ActivationFunctionType.Relu)
                nc.sync.dma_start(out_d[sl, :], o2[:rows, :])

    nc.compile()
    return nc


def kernel(x, Wa, Wb, Wc, attn_w1, attn_b1, attn_w2, rows, cols, vals):
    from concourse.bass_utils import run_bass_kernel_spmd

    x = np.asarray(x, np.float32)
    Wa = np.asarray(Wa, np.float32)
    Wb = np.asarray(Wb, np.float32)
    attn_w1 = np.asarray(attn_w1, np.float32)
    attn_b1 = np.asarray(attn_b1, np.float32)
    attn_w2 = np.asarray(attn_w2, np.float32)
    rows = np.asarray(rows)
    cols = np.asarray(cols)
    vals = np.asarray(vals, np.float32)

    # fold output-combination constants into the SpMM values:
    # out = relu(0.5*spmm0(P) + 0.125*spmm5(D1) + 0.125*spmm6(D2)),
    # D1 = (s1^2-q1)-(s3^2-q3) (bilinear 0.5 folded into 5/6 scales).
    vscale = [0.5, 1.0, 1.0, 1.0, 1.0, 0.125, 0.125]

    # per_core[m][s] = [(tc, tr, tv)] * NT  (col-sorted edges per dest tile)
    per_core = []
    for m in range(NCORE):
        lo, hi = m * NSH, (m + 1) * NSH
        sup = []
        for s in range(7):
            mask = (rows[s] >= lo) & (rows[s] < hi)
            rl = (rows[s][mask] - lo).astype(np.int32)
            cl = cols[s][mask].astype(np.int32)
            vl = (vals[s][mask] * vscale[s]).astype(np.float32)
            sup.append(_sort_tiles(rl, cl, vl))
        per_core.append(sup)

    # shared chunk counts; group chunks by GCH with merged (base,span)
    NCH = [[0] * NT for _ in range(7)]
    GR = [[None] * NT for _ in range(7)]
    for s in range(7):
        for t in range(NT):
            nch = max(1, max((len(per_core[m][s][t][0]) + P - 1) // P
                             for m in range(NCORE)))
            NCH[s][t] = nch
            grs = []
            for g0 in range(0, nch, GCH):
                cnt = min(GCH, nch - g0)
                bases, ends = [], []
                for m in range(NCORE):
                    tc_ = per_core[m][s][t][0]
                    eLo = min(g0 * P, len(tc_))
                    eHi = min((g0 + cnt) * P, len(tc_))
                    if eLo < eHi:
                        bases.append(int(tc_[eLo]))
                        ends.append(int(tc_[eHi - 1]) + 1)
                if not bases:
                    bases, ends = [0], [1]
                b0 = min(bases)
                span0 = min(max(ends) - b0, N - b0)
                assert span0 <= MAX_SPAN, f"merged span {span0} s={s} t={t}"
                grs.append((b0, span0, cnt))
            GR[s][t] = grs
    max_nch = max(NCH[s][t] for s in range(7) for t in range(NT))

    # pack per-core meta: idx [128, nch*8] int16 ; rlv [128, 2*nch] f32
    iota_np = np.tile(np.arange(P, dtype=np.float32), (P, 1)).astype(bf16)
    in_maps = []
    idx_w = ch_w = None
    for m in range(NCORE):
        idx_cols, rlv_cols = [], []
        for s in range(7):
            for t in range(NT):
                tc_, tr_, tv_ = per_core[m][s][t]
                nch = NCH[s][t]
                nv = len(tc_)
                rr = np.zeros(nch * P, np.float32)
                rr[:nv] = tr_
                vv = np.zeros(nch * P, np.float32)
                vv[:nv] = tv_
                rlv_cols.append(rr.reshape(nch, P).T)
                rlv_cols.append(vv.reshape(nch, P).T)
                g0 = 0
                for (b0, span, cnt) in GR[s][t]:
                    eLo = min(g0 * P, nv)
                    eHi = min((g0 + cnt) * P, nv)
                    idx = np.zeros(cnt * P, np.int16)
                    if eLo < eHi:
                        idx[:eHi - eLo] = (tc_[eLo:eHi] - b0).astype(np.int16)
                    idx_cols.append(np.tile(idx.reshape(cnt * 8, 16).T, (8, 1)))
                    g0 += cnt
        idx_all = np.ascontiguousarray(np.concatenate(idx_cols, axis=1))
        rlv_all = np.ascontiguousarray(np.concatenate(rlv_cols, axis=1))
        idx_w, ch_w = idx_all.shape[1], rlv_all.shape[1]
        xt = np.ascontiguousarray(x[m * NSH:(m + 1) * NSH, :].T)
        in_maps.append({
            "xt": xt, "wa": Wa, "wb": Wb, "w1": attn_w1,
            "b1": attn_b1.reshape(1, 32), "w2": attn_w2, "iota": iota_np,
            "idxm": idx_all, "rlvm": rlv_all,
        })

    nc = None
    for fused, chunked_ag in ((True, True), (True, False), (False, False)):
        try:
            nc = _build_program(NCH, GR, idx_w, ch_w, max_nch, fused,
                                chunked_ag)
            break
        except Exception as e:
            print(f"build(fused={fused}, chunked_ag={chunked_ag}) failed: "
                  f"{type(e).__name__}: {e}")
    assert nc is not None, "all program builds failed"
    res = run_bass_kernel_spmd(nc, in_maps, core_ids=list(range(NCORE)))
    out = np.concatenate([res.results[m]["out"] for m in range(NCORE)], axis=0)
    return np.ascontiguousarray(out.astype(np.float32))
